# revision 21
# baseline (speedup 1.0000x reference)
"""Trainium2 Bass kernel for the ChebConv GNN problem
(nn_ChebConvConvolutional): 2x GCNConv + 1x ChebConv(K=3), N=10000 nodes,
E=160000 edges, F=512, celu activations.

Strategy (8 NeuronCores, SPMD):
  * Nodes are sharded 1250/core (padded to 1280). Edges are sharded by
    destination core and grouped into 128-dest tiles; per dest-tile the
    source nodes are deduplicated and the edge weights are baked into dense
    [128 src x 128 dst] one-hot "S" matrices (GCN self-loops folded in as
    edges with value dinv^2, Cheb normalization negated so the scatter
    directly produces lhat).
  * Every graph op is computed aggregate-first: h = celu((A @ x) @ W + b),
    so layer 1 needs no collective (x replicated); layers end with a small
    AllGather of the core's 1280x512 bf16 slice.
  * On device, per dest-tile: dma_gather pulls the (bf16) feature rows of
    the deduped sources; the tensor engine computes
    psumT[f, d] += msgs_chunk[e, f].T @ S[e, d] (feature-major aggregate),
    then the dense GEMM out[n, fo] += aggT_k.T @ W_k (node-major), and
    celu = max(z,0) + min(exp(z)-1, 0) runs on ACT + DVE.
  * ChebConv K=3 is folded into three GEMMs with modified weights:
    out = celu(h2 @ (Wk0-Wk2) + Tx1 @ Wk1 + lhat(Tx1) @ (2*Wk2) + bc).
"""
import numpy as np
import ml_dtypes

import concourse.bacc as bacc
import concourse.mybir as mybir
import concourse.tile as tile
from concourse import library_config
from concourse.bass_utils import run_bass_kernel_spmd
from concourse.tile import add_dep_helper

BF16 = ml_dtypes.bfloat16
FP32 = mybir.dt.float32
BF16D = mybir.dt.bfloat16
I16 = mybir.dt.int16

P = 8            # cores
N = 10000        # nodes
NPC = N // P     # nodes per core
NPAD = 1280      # padded nodes per core
NTOT = NPAD * P
F = 512          # feature width of x / h1 / h2
DOUT = 256
DT = 128         # dests per dest tile
NDT = NPAD // DT # dest tiles per core
KC = F // 128    # contraction chunks (4)
NCH = 2          # AllGather chunks per layer
CH = NPAD // NCH # local rows per AG chunk (256)


# ----------------------------------------------------------------- host prep

def _to_padded_id(n):
    """Global node id -> row in the chunked-AllGather global layout:
    [NCH chunks][P ranks][CH rows]."""
    r = n // NPC
    l = n % NPC
    j = l // CH
    return j * (P * CH) + r * CH + (l % CH)


def _build_edge_tiles(src, dst, val):
    """Shard by dest core, tile by 128 dests, dedup sources per tile.
    Returns (ET [NDT], idx [P, T, 128] int32 padded ids, S [P, T, 128, DT])."""
    per_core = []
    order = np.argsort(dst, kind="stable")
    src, dst, val = src[order], dst[order], val[order]
    core_of = dst // NPC
    core_starts = np.searchsorted(core_of, np.arange(P + 1))
    for c in range(P):
        lo, hi = core_starts[c], core_starts[c + 1]
        s, d, v = src[lo:hi], dst[lo:hi] - c * NPC, val[lo:hi]
        tile_of = d // DT
        tile_starts = np.searchsorted(tile_of, np.arange(NDT + 1))
        groups = []
        for t in range(NDT):
            a, b = tile_starts[t], tile_starts[t + 1]
            st, dl, vt = s[a:b], d[a:b] - t * DT, v[a:b]
            uniq, inv = np.unique(st, return_inverse=True)
            if len(uniq) == 0:
                groups.append((np.zeros(1, np.int64), np.zeros((1, DT), np.float32)))
                continue
            S = np.zeros((len(uniq), DT), np.float32)
            np.add.at(S, (inv, dl), vt)
            groups.append((uniq, S))
        per_core.append(groups)

    ET = [max(max((len(per_core[c][t][0]) + 127) // 128, 1) for c in range(P))
          for t in range(NDT)]
    T = sum(ET)
    off = np.cumsum([0] + ET[:-1])
    idx = np.zeros((P, T, 128), np.int32)
    S_all = np.zeros((P, T, 128, DT), np.float32)
    for c in range(P):
        for t in range(NDT):
            uniq, S = per_core[c][t]
            n = len(uniq)
            o = off[t]
            idx[c, o:o + (n + 127) // 128].reshape(-1)[:n] = _to_padded_id(uniq)
            S_all[c, o:o + (n + 127) // 128].reshape(-1, DT)[:n] = S
    return tuple(ET), idx, S_all


def _idx_dev(idx_core):
    """[T, 128] int32 -> [128, T*8] int16 (wrap 16 partitions, replicate x8)."""
    flat = idx_core.reshape(-1)
    n = len(flat)
    a = np.zeros((16, n // 16), np.int16)
    a[np.arange(n) % 16, np.arange(n) // 16] = flat.astype(np.int16)
    return np.tile(a, (8, 1))


def _s_dev(S_core):
    """[T, 128, DT] -> [128, T*DT] bf16."""
    T = S_core.shape[0]
    return np.ascontiguousarray(
        S_core.transpose(1, 0, 2).reshape(128, T * DT)).astype(BF16)


def _w_dev(W):
    """[F, fo] -> [128, KC*fo] bf16 (chunk k at cols [k*fo, (k+1)*fo))."""
    fi, fo = W.shape
    k = fi // 128
    return np.ascontiguousarray(
        W.reshape(k, 128, fo).transpose(1, 0, 2).reshape(128, k * fo)).astype(BF16)


def _prep(x, edge_index, edge_weight, W1, b1, W2, b2, Wc, bc):
    row = np.asarray(edge_index[0], np.int64)
    col = np.asarray(edge_index[1], np.int64)
    w = np.asarray(edge_weight, np.float32)

    # GCN norm (layers 1 & 2): deg over dest (col) + 1 self loop.
    deg = np.zeros(N, np.float32)
    np.add.at(deg, col, w)
    deg += 1.0
    dinv = (1.0 / np.sqrt(deg)).astype(np.float32)
    g_src = np.concatenate([row, np.arange(N)])
    g_dst = np.concatenate([col, np.arange(N)])
    g_val = np.concatenate([dinv[row] * w * dinv[col], dinv * dinv]).astype(np.float32)

    # Cheb: drop self loops, deg over src (row), negate (lhat = -A_norm).
    keep = row != col
    r0, c0, w0 = row[keep], col[keep], w[keep]
    deg2 = np.zeros(N, np.float32)
    np.add.at(deg2, r0, w0)
    dinv2 = np.where(deg2 > 0, 1.0 / np.sqrt(deg2), 0.0).astype(np.float32)
    c_val = -(dinv2[r0] * w0 * dinv2[c0]).astype(np.float32)

    ETg, idxg, Sg = _build_edge_tiles(g_src, g_dst, g_val)
    ETc, idxc, Sc = _build_edge_tiles(r0, c0, c_val)

    x = np.asarray(x, np.float32)
    x_pad = np.zeros((NTOT, F), BF16)
    x_pad[_to_padded_id(np.arange(N))] = x.astype(BF16)

    Wc = np.asarray(Wc, np.float32)
    com = dict(
        x_bf=x_pad,
        w1=_w_dev(np.asarray(W1, np.float32)),
        w2=_w_dev(np.asarray(W2, np.float32)),
        wa=_w_dev(Wc[0] - Wc[2]),
        wb=_w_dev(Wc[1]),
        wc2=_w_dev(2.0 * Wc[2]),
        ident=np.eye(128, dtype=BF16),
    )
    biases = (np.asarray(b1, np.float32), np.asarray(b2, np.float32),
              np.asarray(bc, np.float32))
    in_maps = []
    for c in range(P):
        m = dict(com)
        m["idxg"] = _idx_dev(idxg[c])
        m["sg"] = _s_dev(Sg[c])
        m["idxc"] = _idx_dev(idxc[c])
        m["sc"] = _s_dev(Sc[c])
        in_maps.append(m)
    return ETg, ETc, biases, in_maps


# ------------------------------------------------------------- bass program

_CACHE = {}


def _build_program(ETg, ETc, has_bias):
    import os
    key = (ETg, ETc, has_bias, os.environ.get("GNN_PHASES", "9"))
    if key in _CACHE:
        return _CACHE[key]
    TG, TC = sum(ETg), sum(ETc)
    ETMAX = max(max(ETg), max(ETc))

    nc = bacc.Bacc("TRN2", target_bir_lowering=False, num_devices=P,
                   num_swdge_queues=4)
    x_bf = nc.dram_tensor("x_bf", [NTOT, F], BF16D, kind="ExternalInput")
    idxg = nc.dram_tensor("idxg", [128, TG * 8], I16, kind="ExternalInput")
    sg = nc.dram_tensor("sg", [128, TG * DT], BF16D, kind="ExternalInput")
    idxc = nc.dram_tensor("idxc", [128, TC * 8], I16, kind="ExternalInput")
    sc = nc.dram_tensor("sc", [128, TC * DT], BF16D, kind="ExternalInput")
    w1 = nc.dram_tensor("w1", [128, KC * F], BF16D, kind="ExternalInput")
    w2 = nc.dram_tensor("w2", [128, KC * F], BF16D, kind="ExternalInput")
    wa = nc.dram_tensor("wa", [128, KC * DOUT], BF16D, kind="ExternalInput")
    wb = nc.dram_tensor("wb", [128, KC * DOUT], BF16D, kind="ExternalInput")
    wc2 = nc.dram_tensor("wc2", [128, KC * DOUT], BF16D, kind="ExternalInput")
    ident = nc.dram_tensor("ident", [128, 128], BF16D, kind="ExternalInput")
    if has_bias:
        brows = nc.dram_tensor("brows", [1, 2 * F + DOUT], FP32, kind="ExternalInput")
    outp = nc.dram_tensor("out", [NPAD, DOUT], FP32, kind="ExternalOutput")

    h1c = nc.dram_tensor("h1c", [NPAD, F], BF16D, kind="Internal")
    h1f = nc.dram_tensor("h1f", [NTOT, F], BF16D, kind="Internal", addr_space="Shared")
    h2c = nc.dram_tensor("h2c", [NPAD, F], BF16D, kind="Internal")
    h2f = nc.dram_tensor("h2f", [NTOT, F], BF16D, kind="Internal", addr_space="Shared")
    t1c = nc.dram_tensor("t1c", [NPAD, F], BF16D, kind="Internal")
    t1f = nc.dram_tensor("t1f", [NTOT, F], BF16D, kind="Internal", addr_space="Shared")

    Exp = mybir.ActivationFunctionType.Exp
    Alu = mybir.AluOpType

    with tile.TileContext(nc) as tc:
        with (
            tc.tile_pool(name="const", bufs=1) as cpool,
            tc.tile_pool(name="keep", bufs=1) as kpool,
            tc.tile_pool(name="msgs", bufs=3) as mpool,
            tc.tile_pool(name="work", bufs=3) as wpool,
            tc.tile_pool(name="psum", bufs=2, space="PSUM") as ppool,
        ):
            lib = nc.gpsimd.load_library(library_config.mlp)

            ig_sb = cpool.tile([128, TG * 8], I16, tag="ig")
            nc.sync.dma_start(ig_sb[:], idxg[:])
            ic_sb = cpool.tile([128, TC * 8], I16, tag="ic")
            nc.sync.dma_start(ic_sb[:], idxc[:])
            id_sb = cpool.tile([128, 128], BF16D, tag="id")
            nc.sync.dma_start(id_sb[:], ident[:])

            sg_sb = cpool.tile([128, TG * DT], BF16D, tag="sg")
            sc_sb = cpool.tile([128, TC * DT], BF16D, tag="sc")
            offg = np.cumsum([0] + list(ETg[:-1]))
            offc = np.cumsum([0] + list(ETc[:-1]))
            for t in range(NDT):
                a, b = offg[t] * DT, (offg[t] + ETg[t]) * DT
                nc.sync.dma_start(sg_sb[:, a:b], sg[:, a:b])
                a, b = offc[t] * DT, (offc[t] + ETc[t]) * DT
                nc.sync.dma_start(sc_sb[:, a:b], sc[:, a:b])

            w1_sb = cpool.tile([128, KC * F], BF16D, tag="w1")
            nc.sync.dma_start(w1_sb[:], w1[:])
            w2_sb = cpool.tile([128, KC * F], BF16D, tag="w2")
            nc.sync.dma_start(w2_sb[:], w2[:])
            wa_sb = cpool.tile([128, KC * DOUT], BF16D, tag="wa")
            nc.sync.dma_start(wa_sb[:], wa[:])
            wb_sb = cpool.tile([128, KC * DOUT], BF16D, tag="wb")
            nc.sync.dma_start(wb_sb[:], wb[:])
            wc2_sb = cpool.tile([128, KC * DOUT], BF16D, tag="wc2")
            nc.sync.dma_start(wc2_sb[:], wc2[:])
            if has_bias:
                br_sb = cpool.tile([1, 2 * F + DOUT], FP32, tag="br")
                nc.sync.dma_start(br_sb[:], brows[:])
                ones_sb = cpool.tile([1, 128], FP32, tag="ones")
                nc.vector.memset(ones_sb[:], 1.0)

            h2keep = kpool.tile([128, NDT, F], BF16D, tag="h2k")
            t1keep = kpool.tile([128, NDT, KC, 128], BF16D, tag="t1k")
            ukeep = kpool.tile([128, NDT, KC, 128], BF16D, tag="uk")

            first_gather = [0]
            qctr = [0]

            def scatter(src_dram, ET, off, idx_sb, s_sb, t):
                """Gather + one-hot matmuls for dest-tile t.
                Returns psum tile [128, KC, 128]: [fi_chunk_part, k, dest].
                The gather is split in two halves on different SWDGE queues so
                descriptor generation runs on two Q7 core pairs in parallel."""
                o = off[t]
                et = ET[t]
                msgs = mpool.tile([128, ETMAX, F], BF16D, tag="msgs")
                nq = min(4, et)
                bounds = [et * i // nq for i in range(nq + 1)]
                for a, b in zip(bounds[:-1], bounds[1:]):
                    if b <= a:
                        continue
                    q = qctr[0] % 4
                    qctr[0] += 1
                    gi = nc.gpsimd.dma_gather(
                        msgs[:, a:b, :], src_dram[:],
                        idx_sb[:, (o + a) * 8:(o + b) * 8],
                        (b - a) * 128, (b - a) * 128, F,
                        single_packet=False, queue_num=q)
                    if first_gather[0] < 4:
                        add_dep_helper(gi.ins, lib.ins,
                                       reason="mlp lib before gather")
                        first_gather[0] += 1
                # S tile as stationary lhsT, msgs streamed as rhs (N=F):
                # ps[d, f] += S[e, d].T @ msgs[e, f]  (node-major aggregate).
                # One matmul per edge tile (vs 4 chunked) minimizes PE
                # instruction count; one contiguous accumulation group.
                ps = ppool.tile([128, F], FP32, tag="psT")
                for g in range(et):
                    nc.tensor.matmul(
                        ps[:],
                        s_sb[:, (o + g) * DT:(o + g + 1) * DT],
                        msgs[:, g, :],
                        start=(g == 0), stop=(g == et - 1))
                return ps

            def celu(z_ps, width, out_ap):
                """out = max(z,0) + min(exp(z)-1, 0); z read from PSUM."""
                e = wpool.tile([128, F], FP32, tag="e")
                nc.scalar.activation(e[:, :width], z_ps, Exp)
                em = wpool.tile([128, F], FP32, tag="em")
                nc.vector.tensor_scalar(
                    em[:, :width], e[:, :width], 1.0, 0.0,
                    Alu.subtract, Alu.min)
                nc.vector.scalar_tensor_tensor(
                    out_ap, z_ps, 0.0, em[:, :width], Alu.max, Alu.add)

            def gemm_bias(z_ps, width, b_off):
                if has_bias:
                    nc.tensor.matmul(
                        z_ps, ones_sb[:],
                        br_sb[:, b_off:b_off + width],
                        start=False, stop=False)

            def allgather_chunk(cin, cout, j):
                nc.gpsimd.collective_compute(
                    "AllGather", Alu.bypass,
                    replica_groups=[list(range(P))],
                    ins=[cin[j * CH:(j + 1) * CH, :]],
                    outs=[cout[j * P * CH:(j + 1) * P * CH, :]])

            def gcn_layer(src_dram, w_sb, dst_dram, keep_tile, b_off, agf):
                for t in range(NDT):
                    ps = scatter(src_dram, ETg, offg, ig_sb, sg_sb, t)
                    agg = wpool.tile([128, F], BF16D, tag="agg")
                    nc.vector.tensor_copy(agg[:], ps[:])
                    tps = ppool.tile([128, KC, 128], BF16D, tag="tps")
                    for k in range(KC):
                        nc.tensor.transpose(
                            tps[:, k, :], agg[:, k * 128:(k + 1) * 128], id_sb[:])
                    aggT = wpool.tile([128, KC, 128], BF16D, tag="aggT")
                    nc.vector.tensor_copy(aggT[:], tps[:])
                    z = ppool.tile([128, F], FP32, tag="z")
                    for k in range(KC):
                        nc.tensor.matmul(
                            z[:], aggT[:, k, :], w_sb[:, k * F:(k + 1) * F],
                            start=(k == 0), stop=(k == KC - 1))
                    gemm_bias(z[:], F, b_off)
                    if keep_tile is None:
                        h = wpool.tile([128, F], BF16D, tag="h")
                        celu(z[:], F, h[:])
                        nc.sync.dma_start(dst_dram[t * 128:(t + 1) * 128, :], h[:])
                    else:
                        celu(z[:], F, keep_tile[:, t, :])
                        nc.sync.dma_start(dst_dram[t * 128:(t + 1) * 128, :],
                                          keep_tile[:, t, :])
                    if agf is not None and (t * 128 + 128) % CH == 0:
                        allgather_chunk(dst_dram, agf, (t * 128 + 128) // CH - 1)

            import os
            PH = int(os.environ.get("GNN_PHASES", "9"))

            # ---- layer 1: h1 = celu((Ag @ x) @ W1 + b1)
            gcn_layer(x_bf, w1_sb, h1c, None, 0, h1f if PH >= 2 else None)

            # ---- layer 2: h2 = celu((Ag @ h1) @ W2 + b2); keep h2 on chip
            if PH >= 3:
                gcn_layer(h1f, w2_sb, h2c, h2keep, F, h2f if PH >= 4 else None)

            # ---- cheb pass 1: Tx1 = lhat(h2) (feature-major kept + node-major
            #      transposed out for the next gather)
            if PH >= 5:
                for t in range(NDT):
                    ps = scatter(h2f, ETc, offc, ic_sb, sc_sb, t)
                    tnm = wpool.tile([128, F], BF16D, tag="h")
                    nc.vector.tensor_copy(tnm[:], ps[:])
                    nc.sync.dma_start(t1c[t * 128:(t + 1) * 128, :], tnm[:])
                    tps = ppool.tile([128, KC, 128], BF16D, tag="tps")
                    for k in range(KC):
                        nc.tensor.transpose(
                            tps[:, k, :], tnm[:, k * 128:(k + 1) * 128], id_sb[:])
                    nc.vector.tensor_copy(t1keep[:, t, :, :], tps[:])
                    if (t * 128 + 128) % CH == 0:
                        allgather_chunk(t1c, t1f, (t * 128 + 128) // CH - 1)

            # ---- cheb pass 2: U = lhat(Tx1), kept feature-major
            if PH >= 6:
                for t in range(NDT):
                    ps = scatter(t1f, ETc, offc, ic_sb, sc_sb, t)
                    unm = wpool.tile([128, F], BF16D, tag="h")
                    nc.vector.tensor_copy(unm[:], ps[:])
                    tps = ppool.tile([128, KC, 128], BF16D, tag="tps")
                    for k in range(KC):
                        nc.tensor.transpose(
                            tps[:, k, :], unm[:, k * 128:(k + 1) * 128], id_sb[:])
                    nc.vector.tensor_copy(ukeep[:, t, :, :], tps[:])

            # ---- cheb output:
            # out = celu(h2 @ Wa + Tx1 @ Wb + U @ Wc2 + bc)
            for t in range(NDT if PH >= 7 else 0):
                hps = ppool.tile([128, KC, 128], BF16D, tag="tps")
                for k in range(KC):
                    nc.tensor.transpose(
                        hps[:, k, :], h2keep[:, t, k * 128:(k + 1) * 128], id_sb[:])
                h2T = wpool.tile([128, KC, 128], BF16D, tag="h2T")
                nc.vector.tensor_copy(h2T[:], hps[:])
                zo = ppool.tile([128, DOUT], FP32, tag="z")
                for k in range(KC):
                    nc.tensor.matmul(
                        zo[:], h2T[:, k, :], wa_sb[:, k * DOUT:(k + 1) * DOUT],
                        start=(k == 0), stop=False)
                for k in range(KC):
                    nc.tensor.matmul(
                        zo[:], t1keep[:, t, k, :], wb_sb[:, k * DOUT:(k + 1) * DOUT],
                        start=False, stop=False)
                for k in range(KC):
                    nc.tensor.matmul(
                        zo[:], ukeep[:, t, k, :], wc2_sb[:, k * DOUT:(k + 1) * DOUT],
                        start=False, stop=(k == KC - 1))
                gemm_bias(zo[:], DOUT, 2 * F)
                of = wpool.tile([128, DOUT], FP32, tag="of")
                celu(zo[:], DOUT, of[:])
                nc.sync.dma_start(outp[t * 128:(t + 1) * 128, :], of[:])

    nc.compile()
    _CACHE[key] = nc
    return nc


# ------------------------------------------------------------------- driver

def _run(inputs, trace=False, tmpdir=None):
    ETg, ETc, biases, in_maps = _prep(**inputs)
    has_bias = any(np.any(b != 0) for b in biases)
    if has_bias:
        brow = np.concatenate(biases).astype(np.float32)[None, :]
        for m in in_maps:
            m["brows"] = brow
    nc = _build_program(ETg, ETc, has_bias)
    res = run_bass_kernel_spmd(nc, in_maps, core_ids=list(range(P)),
                               trace=trace, tmpdir=tmpdir)
    out = np.concatenate(
        [res.results[c]["out"][:NPC] for c in range(P)], axis=0)
    return out.astype(np.float32), res


def kernel(**inputs) -> np.ndarray:
    out, _ = _run(inputs)
    return out
